# revision 16
# baseline (speedup 1.0000x reference)
"""Trainium2 Bass kernel for the differentiable compressor.

Algorithm
---------
The smoothing recurrence  s_t = a_t s_{t-1} + (1-a_t) v_t,
a_t = A_AT if v_t > s_{t-1} else A_REL  is solved by lagged policy
iteration in relative coordinates r_t = s_t - v_t:
    r_t = a_t * (r_{t-1} + delta_t),   delta_t = v_{t-1} - v_t
with delta precomputed once.  Key identity: since a_t > 0, the next
sweep's mode  m_t = [r_{t-1} + delta_t < 0]  equals  [r_t < 0], so each
sweep's coefficients come from a Sign + affine pair on the Scalar
engine - the Vector engine runs scans back to back.  4 sweeps with
lagged chunk carries land at ~7.6e-4 relative error vs the 2e-2 gate.

Layout per core: 2 batch rows x 441000 samples -> [126 x 7000], 63
time-chunks per row; chunk carries live in an extra leading column of
the trajectory tile, refreshed between sweeps by two tiny SBUF->SBUF
DMAs.

Gain: the knee's eps-smoothing (width 0.01 dB) is dropped and the two
gates collapse to
    g = min(2CUP relu(-(w+KN)), UPR-cupK) - |2CDN| relu(w-KN)
        - Kbar sign(w) + C0,    w = level - th
(exact outside the 0.1 dB knee interior; ~1.7e-4 overall).  The -th
shift rides in the per-partition bias operands of the three Scalar-
engine ops, Relu/Relu/Sign/Exp/Abs/Ln all live in one activation
table (explicitly pinned, zero mid-kernel table loads), and the Vector
engine does two fused scalar_tensor_tensors + the y multiply per
quarter, pipelined against ACT and the output DMA.

Sharding: pure data parallel, batch 16 -> 2 rows on each of 8 cores.
"""
import sys
import types
import numpy as np

# ---------------- constants (natural-log units) ----------------
SR = 44100.0
A_AT = float(np.exp(-1.0 / (10.0 * SR / 1000.0)))     # attack coeff
A_REL = float(np.exp(-1.0 / (100.0 * SR / 1000.0)))   # release coeff
DA = A_AT - A_REL
AMID = (A_AT + A_REL) / 2.0
CNAT = float(np.log(10.0) / 20.0)                     # dB -> nat
KN = 0.1 * CNAT                                       # knee
M2CDN = 1.0 - 1.0 / 66.7                              # |2*CDN|
M2CUP = 1.0 - 0.1                                     # 2*CUP
CDNK = M2CDN * KN
CUPK = M2CUP * KN
UPR = 36.0 * CNAT
UPRP = UPR - CUPK
C0 = (CUPK - CDNK) / 2.0
KBAR = (CUPK + CDNK) / 2.0
TMIN, TMAX = -40.0, 0.0

B, N = 16, 441000
NCORES = 8
ROWS = 2           # batch rows per core
NCH = 63           # chunks per row
P = ROWS * NCH     # 126 partitions
L = N // NCH       # 7000 chunk length
H = L // 2         # half width
Q = L // 4         # quarter width
NS = 8             # x DMA / start-phase chunks
CW = L // NS

N_SWEEPS = 4


def _install_ntff_hook():
    """Inject the missing antenv.axon_hooks so trace=True profiling works."""
    try:
        import antenv
        if "antenv.axon_hooks" not in sys.modules:
            m = types.ModuleType("antenv.axon_hooks")
            m._hook = None
            def _set(h, _m=m): _m._hook = h
            def _get(_m=m): return _m._hook
            m.set_axon_ntff_profile_hook = _set
            m.get_axon_ntff_profile_hook = _get
            sys.modules["antenv.axon_hooks"] = m
            antenv.axon_hooks = m
            from trn_agent_boot.trn_boot import _ntff_profile_via_ctypes
            _set(_ntff_profile_via_ctypes("/opt/axon/libaxon_pjrt.so"))
    except Exception:
        pass


def build_nc():
    import concourse.bacc as bacc
    import concourse.mybir as mybir
    from concourse.tile import TileContext
    from concourse.alu_op_type import AluOpType as Op
    AF = mybir.ActivationFunctionType
    f32 = mybir.dt.float32

    nc = bacc.Bacc("TRN2", target_bir_lowering=False, debug=False)
    x_d = nc.dram_tensor("x", [P, L], f32, kind="ExternalInput")
    # per-partition scalars: [-th, dep, dep*C0, bup, bdn]
    sc_d = nc.dram_tensor("sc", [P, 5], f32, kind="ExternalInput")
    # carry shift matrix: shm[src, dst] = 1 iff dst == src+1 and dst % NCH != 0
    shm_d = nc.dram_tensor("shm", [P, P], f32, kind="ExternalInput")
    y_d = nc.dram_tensor("y", [P, L], f32, kind="ExternalOutput")

    with TileContext(nc) as tc:
        with tc.tile_pool(name="pool", bufs=1) as pool, \
             tc.psum_pool(name="pp", bufs=1) as pp:
            tx = pool.tile([P, L], f32)        # x (kept for final multiply)
            tv = pool.tile([P, L], f32)        # v; post: A3 scratch
            tD = pool.tile([P, L], f32)        # delta; post: A1/u/G/gain
            tse = pool.tile([P, L], f32)       # r trajectory
            ta = pool.tile([P, L], f32)        # modes -> a; post: A2/p/y
            tsc = pool.tile([P, 5], f32)
            tshm = pool.tile([P, P], f32)      # carry shift matrix
            tcar = pp.tile([P, 1], f32)        # shifted carries (PSUM)
            tvL = pool.tile([P, 1], f32)       # v[:, L-1] (early)
            tcol = pool.tile([P, 1], f32)      # prev-chunk-end v column
            # constant columns for activation bias operands
            tcst = pool.tile([P, 2], f32)
            c1e8, cmid = (tcst[:, i:i + 1] for i in range(2))
            nc.vector.memset(c1e8, 1e-8)
            nc.vector.memset(cmid, AMID)
            nth, dep, bx, bup, bdn = (tsc[:, i:i + 1] for i in range(5))

            # pin the act table holding abs/ln/identity/relu/sign/exp
            atl = mybir.InstLoadActFuncSet(
                name=nc.get_next_instruction_name(), ins=[], outs=[],
                act_func_set_id=6)
            nc.scalar.add_instruction(atl)

            # last x column first: unblocks the cross-chunk delta column
            nc.sync.dma_start(tvL[:], x_d[:, L - 1:L])
            nc.sync.dma_start(tx[:, 0:CW], x_d[:, 0:CW])
            nc.sync.dma_start(tx[:, CW:2 * CW], x_d[:, CW:2 * CW])
            nc.sync.dma_start(tsc[:], sc_d[:])
            nc.sync.dma_start(tshm[:], shm_d[:])
            for j in range(2, NS):
                sl = slice(j * CW, (j + 1) * CW)
                nc.sync.dma_start(tx[:, sl], x_d[:, sl])

            # v[:, L-1] = ln(|x_L-1| + 1e-8), then shift across partitions
            nc.scalar.activation(tvL[:], tvL[:], AF.Abs, bias=0.0, scale=1.0)
            nc.scalar.activation(tvL[:], tvL[:], AF.Ln, bias=c1e8, scale=1.0)
            nc.sync.dma_start(tcol[1:NCH, 0:1], tvL[0:NCH - 1, 0:1])
            nc.sync.dma_start(tcol[NCH + 1:P, 0:1], tvL[NCH:P - 1, 0:1])

            # chunked: v = ln(|x|+1e-8); delta = v_{t-1} - v_t; it-0 modes
            # m0 = [delta < 0], a0 = A_REL + DA*m0 (both on DVE, hidden
            # under the DMA/Ln stream).  Chunk 0's col 0 is cross-chunk:
            # its delta/a ops are emitted right after chunk 0 (the Vector
            # engine runs its queue in order - emitting them any later
            # would gate the first scan on the last chunk).
            for j in range(NS):
                sl = slice(j * CW, (j + 1) * CW)
                nc.scalar.activation(tv[:, sl], tx[:, sl], AF.Abs, bias=0.0, scale=1.0)
                nc.scalar.activation(tv[:, sl], tv[:, sl], AF.Ln, bias=c1e8, scale=1.0)
                lo = j * CW
                s_in = slice(lo if j else 1, (j + 1) * CW)
                s_sh = slice((lo - 1) if j else 0, (j + 1) * CW - 1)
                nc.vector.tensor_tensor(tD[:, s_in], tv[:, s_sh], tv[:, s_in],
                                        Op.subtract)
                nc.vector.tensor_scalar(ta[:, s_in], tD[:, s_in], 0.0, DA,
                                        op0=Op.is_lt, op1=Op.mult)
                nc.vector.tensor_scalar(ta[:, s_in], ta[:, s_in], A_REL, None,
                                        op0=Op.add)
                if j == 0:
                    # col-0: rows 0 and NCH have no predecessor -> delta 0
                    nc.sync.dma_start(tcol[0:1, 0:1], tv[0:1, 0:1])
                    nc.sync.dma_start(tcol[NCH:NCH + 1, 0:1],
                                      tv[NCH:NCH + 1, 0:1])
                    nc.vector.tensor_tensor(tD[:, 0:1], tcol[:, 0:1],
                                            tv[:, 0:1], Op.subtract)
                    nc.vector.tensor_scalar(ta[:, 0:1], tD[:, 0:1], 0.0, DA,
                                            op0=Op.is_lt, op1=Op.mult)
                    nc.vector.tensor_scalar(ta[:, 0:1], ta[:, 0:1], A_REL, None,
                                            op0=Op.add)

            # ---------------- sweeps ----------------
            # DVE runs scans back to back; the next sweep's coefficients
            # a = AMID - (DA/2)*sign(r) come from a Sign+Identity pair on
            # the Scalar engine, hidden under the opposite half's scan.
            # Chunk carries: the Tensor engine multiplies the end column by
            # a shifted-identity matrix (zero rows at chunk-0 positions)
            # into PSUM, which the next h1 scan reads as its initial state
            # - much lower latency than an SBUF->SBUF partition-shift DMA.
            for k in range(N_SWEEPS):
                last = k == N_SWEEPS - 1
                for h in range(2):
                    sl = slice(h * H, (h + 1) * H)
                    init = 0.0 if k == 0 and h == 0 else (
                        tcar[:, 0:1] if h == 0 else tse[:, H - 1:H])
                    nc.vector.tensor_tensor_scan(
                        tse[:, sl], tD[:, sl], ta[:, sl], init,
                        op0=Op.add, op1=Op.mult)
                    if not last:
                        nc.scalar.activation(ta[:, sl], tse[:, sl], AF.Sign,
                                             bias=0.0, scale=1.0)
                        nc.scalar.activation(ta[:, sl], ta[:, sl], AF.Identity,
                                             bias=cmid, scale=-DA / 2.0)
                if not last:
                    nc.tensor.matmul(tcar[:], tshm[:], tse[:, L - 1:L],
                                     start=True, stop=True)

            # ---------------- tail: W = r + v, then gain ----------------
            #   A1 = relu(-M2CUP*W + bup);  A2 = relu(M2CDN*W + bdn)
            #   A3 = sign(W - th);  p = -KBAR*A3 - A2
            #   G = min(UPRP, A1) + p;  gain = exp(dep*G + dep*C0)
            for q in range(4):
                sl = slice(q * Q, (q + 1) * Q)
                w = tse[:, sl]
                nc.vector.tensor_tensor(w, w, tv[:, sl], Op.add)
                nc.scalar.activation(tD[:, sl], w, AF.Relu,
                                     bias=bup, scale=-M2CUP)
                nc.scalar.activation(ta[:, sl], w, AF.Relu,
                                     bias=bdn, scale=M2CDN)
                nc.scalar.activation(tv[:, sl], w, AF.Sign, bias=nth, scale=1.0)
                nc.vector.scalar_tensor_tensor(
                    ta[:, sl], tv[:, sl], -KBAR, ta[:, sl],
                    op0=Op.mult, op1=Op.subtract)
                nc.vector.scalar_tensor_tensor(
                    tD[:, sl], tD[:, sl], UPRP, ta[:, sl],
                    op0=Op.min, op1=Op.add)
                nc.scalar.activation(tD[:, sl], tD[:, sl], AF.Exp,
                                     bias=bx, scale=dep[:, 0:1])
                if q < 3:
                    nc.vector.tensor_tensor(ta[:, sl], tD[:, sl], tx[:, sl], Op.mult)
                    nc.sync.dma_start(y_d[:, sl], ta[:, sl])
                else:
                    for e in range(2):
                        se = slice(q * Q + e * (Q // 2), q * Q + (e + 1) * (Q // 2))
                        nc.vector.tensor_tensor(ta[:, se], tD[:, se], tx[:, se],
                                                Op.mult)
                        nc.sync.dma_start(y_d[:, se], ta[:, se])

    nc.compile()
    return nc


_NC = None


def _get_nc():
    global _NC
    if _NC is None:
        _NC = build_nc()
    return _NC


def make_in_maps(x, threshold, depth):
    th_nat = ((TMIN + threshold.astype(np.float32) * (TMAX - TMIN)) *
              np.float32(CNAT)).astype(np.float32)           # [16,1]
    dep = depth.astype(np.float32)
    # shm[src, dst] = 1 iff dst == src+1 and dst is not a chunk-0 row
    shm = np.zeros((P, P), np.float32)
    for src in range(P - 1):
        dst = src + 1
        if dst % NCH != 0:
            shm[src, dst] = 1.0
    in_maps = []
    for i in range(NCORES):
        xs = np.ascontiguousarray(x[ROWS * i:ROWS * (i + 1)]).reshape(P, L)
        th_c = np.repeat(th_nat[ROWS * i:ROWS * (i + 1), 0], NCH)    # [P]
        dep_c = np.repeat(dep[ROWS * i:ROWS * (i + 1), 0], NCH)
        sc = np.stack([-th_c,
                       dep_c,
                       dep_c * np.float32(C0),
                       np.float32(M2CUP) * (th_c - np.float32(KN)),
                       np.float32(-M2CDN) * (th_c + np.float32(KN))],
                      axis=1)
        in_maps.append({"x": xs.astype(np.float32),
                        "sc": np.ascontiguousarray(sc, np.float32),
                        "shm": shm})
    return in_maps


def kernel(x, threshold, depth):
    _install_ntff_hook()
    from concourse.bass_utils import run_bass_kernel_spmd
    nc = _get_nc()
    x = np.asarray(x, np.float32)
    in_maps = make_in_maps(x, np.asarray(threshold), np.asarray(depth))
    res = run_bass_kernel_spmd(nc, in_maps, core_ids=list(range(NCORES)))
    y = np.empty((B, N), np.float32)
    for i in range(NCORES):
        y[ROWS * i:ROWS * (i + 1)] = np.asarray(res.results[i]["y"]).reshape(ROWS, N)
    return y
